# revision 35
# baseline (speedup 1.0000x reference)
"""Trainium2 Bass kernel for nn_AcousticRadianceTransfer_PatchToPatch.

Strategy
--------
The 8-bounce frequency-domain loop  cur <- SpMM(vals; row,col) @ (cur * D)
is diagonal in frequency, so the 801 rfft bins are sharded over the 8 cores
(104 padded bins each) with ZERO inter-core communication.  The final output
only needs w @ total, so the big irfft collapses to a single 1600-point
irfft of the per-bounce detection spectra on the host; the R0 term reduces
to w @ initial_radiance in the time domain.

Per core:
  Phase A:  R0 = x @ W  (DFT as fp16 TensorE matmul; fsm window folded into
            W; x pre-transposed on host), written to SBUF `acc` (fp16).
  8 bounces: tmp = acc * D  ->  HBM  (fp16, rows padded to 512B pitch);
            SpMM as slot-structured dma_gather (rows degree-sorted so each
            "slot s" gather covers the prefix of rows with degree > s) +
            VectorE  acc += v * C;  per-bounce detection matvec
            e_b = w @ acc on TensorE into PSUM.
  Gathers move the 416B payload (208 fp16) from 512B-pitch rows in
  4096-index calls, staged 4-deep with one buffer per SWDGE queue
  (measured: call size and staging depth dominate SWDGE gather rate;
  82 GB/s at 5120-idx/2-deep -> 165+ GB/s at 4096-idx/4-deep).  The
  per-edge weight v is expanded along the (half-)frequency axis by the
  otherwise-idle Activation engine (v is real, so re/im halves share one
  expansion); both VectorE passes then run packed fp16 at 2x.
  Magnitudes are kept in fp16 range by per-bounce scales sigma_b folded
  into the edge weights (exact algebra, host-compensated).

Everything data-dependent (degree sort, slot plan, scales) is host-side
preprocessing; the device graph is compiled per problem instance.
"""

import math
import os
import sys

import numpy as np

if "/opt/trn_rl_repo" not in sys.path:
    sys.path.insert(0, "/opt/trn_rl_repo")

import concourse.bacc as bacc
import concourse.bass as bass
import concourse.mybir as mybir
from concourse.bass_utils import run_bass_kernel_spmd
from concourse.library_config import mlp

# ---------------- problem constants (hardcoded per contract) ----------------
SR = 16000
GAMMA = 1e-3
NB = 8                 # bounces
NRAD = 24000
L = 1600
F = L // 2 + 1         # 801
N_CORES = 8
FC = 104               # complex freqs per core (8*104 = 832 >= 801)
FPAD = N_CORES * FC    # 832
E = 2 * FC             # 208 fp16 values per radiance row slice  [re(104)|im(104)]
NROW = 24064           # 24000 padded to 188*128
NT = NROW // 128       # 188 chunk-cols
STRIDE = 256           # fp16 elems per tmp row in HBM (512B pitch)
KC = 13                # 1600 -> 13 chunks of 128 (last zero-padded)
GCC = 32               # gather staging: chunk-cols per call (<= 4096 idxs)
NBUF = 4               # gather staging depth (one buffer per SWDGE queue)
DCC = 16               # D-mult chunk-cols  (188 = 11*16 + 12)
DT16 = mybir.dt.float16
DT32 = mybir.dt.float32
DTI16 = mybir.dt.int16

_D_CHUNKS = [(i * DCC, min(DCC, NT - i * DCC)) for i in range((NT + DCC - 1) // DCC)]
NDC = len(_D_CHUNKS)   # 12


def _patch_dma_gather():
    """Relax bass's 256B elem-size assert (only the HBM row STRIDE must be a
    multiple of 256B; the payload length is arbitrary — verified on HW)."""
    import inspect
    import textwrap

    if getattr(bass.BassGpSimd.dma_gather, "_relaxed", False):
        return
    src = inspect.getsource(bass.BassGpSimd.dma_gather)
    src = src.replace(
        "elem_size_bytes > 0 and elem_size_bytes % 256 == 0",
        "elem_size_bytes > 0 and elem_size_bytes % 2 == 0",
    )
    ns = dict(bass.BassGpSimd.dma_gather.__globals__)
    exec(textwrap.dedent(src), ns)
    fn = ns["dma_gather"]
    fn._relaxed = True
    bass.BassGpSimd.dma_gather = fn


# ===========================================================================
# host preprocessing
# ===========================================================================

def _plan_calls(ndeg):
    """Static gather-call plan from the (descending) degree array."""
    maxdeg = int(ndeg.max())
    counts = np.bincount(ndeg, minlength=maxdeg + 2)
    # Ns[s] = #rows with degree > s  (rows sorted desc -> prefix property)
    tail = np.cumsum(counts[::-1])[::-1]
    calls = []
    for s in range(maxdeg):
        Ns = int(tail[s + 1])
        if Ns <= 0:
            break
        ncc_s = (Ns + 127) // 128
        for cc0 in range(0, ncc_s, GCC):
            ncc = min(GCC, ncc_s - cc0)
            base = cc0 * 128
            nidx = min(Ns - base, ncc * 128)
            calls.append(dict(slot=s, cc0=cc0, ncc=ncc, nidx=nidx,
                              first=(s == 0)))
    return calls


def _pack_idx(idx_vals, icc):
    """int16 [128, icc]: wrapped in 16 partitions, replicated across 8 Q7 cores."""
    n = len(idx_vals)
    arr = np.full((icc, 16), -1, np.int16)
    arr.reshape(-1)[:n] = idx_vals             # position i -> [i//16, i%16]
    return np.tile(arr.T, (8, 1))              # -> [p=i%16, col=i//16]


def preprocess(inputs):
    """All host-side data preparation.  Returns (plan, in_maps, post)."""
    a = np.asarray(inputs["absorption_coefficient"], np.float64)
    s = np.asarray(inputs["scattering_coefficient"], np.float64)
    x = np.asarray(inputs["initial_radiance"], np.float32)
    wdet = np.asarray(inputs["detection_weights"], np.float64)
    env = np.asarray(inputs["envelope"], np.float64)
    basis = np.asarray(inputs["sparse_kernel_basis"], np.float64)
    row = np.asarray(inputs["sparse_kernel_row"]).astype(np.int64)
    col = np.asarray(inputs["sparse_kernel_col"]).astype(np.int64)
    refl = np.asarray(inputs["sparse_kernel_reflector_id"]).astype(np.int64)
    dly = np.asarray(inputs["delay_samples"], np.float64)

    log_gamma = math.log(GAMMA)
    t = np.arange(L) / SR
    fsm = np.exp(log_gamma * t)

    vals = (s[:, refl] * basis).sum(0) * a[refl]          # [nnz]

    # --- degree sort / relabel (dummy edge for degree-0 rows) ---
    deg = np.bincount(row, minlength=NRAD)
    zrows = np.nonzero(deg == 0)[0]
    if len(zrows):
        row = np.concatenate([row, zrows])
        col = np.concatenate([col, np.zeros(len(zrows), np.int64)])
        vals = np.concatenate([vals, np.zeros(len(zrows))])
        deg = np.bincount(row, minlength=NRAD)
    order = np.argsort(-deg, kind="stable")               # old rows, deg desc
    new_of_old = np.empty(NRAD, np.int64)
    new_of_old[order] = np.arange(NRAD)
    nrow = new_of_old[row]
    ncol = new_of_old[col]
    ndeg = deg[order]                                     # degree by new label

    # --- per-bounce scales from the exact DC trajectory (upper bound) ---
    D0 = np.exp(log_gamma * dly / SR)
    cur_dc = (x.astype(np.float64) * fsm).sum(1)
    sig = []
    for b in range(NB):
        nxt = np.zeros(NRAD)
        np.add.at(nxt, row, vals * (D0 * cur_dc)[col])
        g = max(nxt.max(), 1e-30) / 256.0
        sig.append(g)
        cur_dc = nxt / g
    sig = np.array(sig)
    alpha = np.cumprod(sig)

    # --- slot structure: edges sorted by (new row, position) ---
    nnz = len(vals)
    eorder = np.lexsort((np.arange(nnz), nrow))
    erow = nrow[eorder]
    ecol = ncol[eorder]
    ev = vals[eorder]
    starts = np.zeros(NRAD + 1, np.int64)
    np.cumsum(np.bincount(erow, minlength=NRAD), out=starts[1:])

    calls = _plan_calls(ndeg)

    idx_blocks = []
    v_blocks = [[] for _ in range(NB)]
    for c in calls:
        sslot, base, nidx, ncc = c["slot"], c["cc0"] * 128, c["nidx"], c["ncc"]
        eidx = starts[base:base + nidx] + sslot
        icc = (nidx + 15) // 16
        c["icc"] = icc
        idx_blocks.append(_pack_idx(ecol[eidx].astype(np.int16), icc))
        vraw = np.zeros(ncc * 128, np.float64)
        vraw[:nidx] = ev[eidx]
        vgrid = vraw.reshape(ncc, 128).T                  # [128, ncc]
        for b in range(NB):
            v_blocks[b].append((vgrid / sig[b]).astype(np.float16))
    idx_all = np.concatenate(idx_blocks, axis=1)          # [128, IT]
    vv_all = np.stack([np.concatenate(vb, axis=1) for vb in v_blocks])  # [NB,128,VT]

    # --- DFT operands ---
    xp = np.zeros((NROW, KC * 128), np.float16)
    xp[new_of_old, :L] = x.astype(np.float16)
    xT = np.ascontiguousarray(
        xp.reshape(NT, 128, KC, 128).transpose(0, 3, 2, 1)
    ).reshape(NT, 128, KC * 128)                          # [g, time, k*128+r]

    k_all = np.arange(FPAD)
    # W[t, k] = fsm[t] * exp(-2i pi t k / L); zero for k >= F and t >= L
    Wfull = np.zeros((KC * 128, FPAD), np.complex128)
    Wfull[:L, :F] = np.exp(-2j * np.pi * np.outer(np.arange(L), k_all[:F]) / L) \
        * fsm[:, None]

    Dc = np.exp(log_gamma * dly / SR)[:, None] * \
        np.exp(-2j * np.pi * np.outer(dly, k_all) / L)    # [NRAD, FPAD]
    Dc[:, F:] = 0.0

    wperm = np.zeros(NROW)
    wperm[new_of_old] = wdet
    wdet_sb = wperm.reshape(NT, 128).T.astype(np.float16)  # [128, NT]

    in_maps = []
    for cidx in range(N_CORES):
        ks = slice(cidx * FC, (cidx + 1) * FC)
        Wc = np.empty((KC, 128, E), np.float16)
        Wslice = Wfull[:, ks]
        Wc[:, :, :FC] = Wslice.real.reshape(KC, 128, FC)
        Wc[:, :, FC:] = Wslice.imag.reshape(KC, 128, FC)
        Wc = np.ascontiguousarray(Wc.transpose(1, 0, 2))  # [128, KC, E]
        Dslice = Dc[:, ks]
        Dacc = np.zeros((128, NT, E), np.float16)
        Dre = np.zeros((NROW, FC)); Dim = np.zeros((NROW, FC))
        Dre[new_of_old] = Dslice.real
        Dim[new_of_old] = Dslice.imag
        Dacc[:, :, :FC] = Dre.reshape(NT, 128, FC).transpose(1, 0, 2)
        Dacc[:, :, FC:] = Dim.reshape(NT, 128, FC).transpose(1, 0, 2)
        in_maps.append({
            "xT": xT,
            "W": Wc,
            "wdet": wdet_sb,
            "Dsp": Dacc.astype(np.float16),
            "vv": vv_all,
            "idx": idx_all,
        })

    post = dict(alpha=alpha, fsm=fsm, env=env,
                e0=float(0.0) * np.zeros(1),  # placeholder
                )
    post["e0"] = wdet @ x.astype(np.float64)              # [L]
    plan = dict(calls=calls, IT=idx_all.shape[1], VT=vv_all.shape[2])
    return plan, in_maps, post


def postprocess(results, post):
    """Assemble echogram from per-core per-bounce detection spectra."""
    alpha, fsm, env, e0 = post["alpha"], post["fsm"], post["env"], post["e0"]
    e_spec = np.zeros(FPAD, np.complex128)
    for cidx in range(N_CORES):
        eb = np.asarray(results[cidx]["e"], np.float64).reshape(NB, 256)
        ks = slice(cidx * FC, (cidx + 1) * FC)
        acc = np.zeros(FC, np.complex128)
        for b in range(NB):
            acc += alpha[b] * (eb[b, :FC] + 1j * eb[b, FC:E])
        e_spec[ks] = acc
    echo_b = np.fft.irfft(e_spec[:F], n=L) / fsm
    echo = (e0 + echo_b) * np.exp(env)
    return echo.astype(np.float32)


# ===========================================================================
# device graph
# ===========================================================================

def build_nc(plan, debug=None):
    """debug: None (full), "r0" (phase A only), "tmp1" (+tail0), "cur1"
    (+bounce1 SpMM), "cur8" (all SpMM, dump acc after bounce 8)."""
    calls = plan["calls"]
    ncalls = len(calls)
    IT, VT = plan["IT"], plan["VT"]

    # how much of the pipeline to emit
    if debug == "r0":
        n_tails, n_spmm, emit_e = 0, 0, False
    elif debug == "tmp1":
        n_tails, n_spmm, emit_e = 1, 0, False
    elif debug == "cur1":
        n_tails, n_spmm, emit_e = 1, 1, False
    elif debug == "cur8":
        n_tails, n_spmm, emit_e = NB, NB, False
    elif debug == "b4":
        n_tails, n_spmm, emit_e = 4, 4, True
    elif debug == "b1":
        n_tails, n_spmm, emit_e = 1, 1, True
    elif debug == "b0":
        n_tails, n_spmm, emit_e = 1, 0, True
    elif debug == "b16":
        n_tails, n_spmm, emit_e = 16, 16, True
    else:
        n_tails, n_spmm, emit_e = NB, NB, True
    # timing-isolation variants: g8 = gathers only; m8 = all but gathers
    do_gather = debug != "m8"
    do_mac = debug != "g8"
    if debug == "g8":
        n_tails, n_spmm, emit_e = 1, NB, False
    elif debug == "m8":
        n_tails, n_spmm, emit_e = NB, NB, True

    # gather experiment knobs (env): payload elems per row, #SWDGE queues
    GELEM = int(os.environ.get("GELEM", "208"))
    GQ = int(os.environ.get("GQ", "4"))
    if (GELEM * 2) % 256:
        _patch_dma_gather()

    nc = bacc.Bacc("TRN2", num_swdge_queues=GQ,
                   dynamic_dma_scratch_size=int(os.environ.get("DSCR", "32768")))

    xT_e = nc.declare_dram_parameter("xT", [NT, 128, KC * 128], DT16, isOutput=False)
    W_e = nc.declare_dram_parameter("W", [128, KC, E], DT16, isOutput=False)
    wdet_e = nc.declare_dram_parameter("wdet", [128, NT], DT16, isOutput=False)
    Dsp_e = nc.declare_dram_parameter("Dsp", [128, NT, E], DT16, isOutput=False)
    vv_e = nc.declare_dram_parameter("vv", [NB, 128, VT], DT16, isOutput=False)
    idx_e = nc.declare_dram_parameter("idx", [128, IT], DTI16, isOutput=False)
    e_out = None
    if emit_e:
        e_out = nc.declare_dram_parameter("e", [1, NB * 256], DT32, isOutput=True)
    dbg_e = None
    if debug in ("r0", "cur1", "cur8"):
        dbg_e = nc.declare_dram_parameter("dbg", [128, NT, E], DT16, isOutput=True)
    elif debug == "tmp1":
        dbg_e = nc.declare_dram_parameter("dbg", [NROW, STRIDE], DT16, isOutput=True)
    elif debug == "g8":
        dbg_e = nc.declare_dram_parameter("dbg", [128, 16], DT16, isOutput=True)

    tmp = nc.dram_tensor("tmp", [NROW, STRIDE], DT16)

    ioff = 0
    voff = 0
    for c in calls:
        c["ioff"] = ioff
        c["voff"] = voff
        ioff += c["icc"]
        voff += c["ncc"]
    assert ioff == IT and voff == VT

    # ---- per-chunk detection release: process D-chunks in descending-t ----
    # order; chunk rows get their last MAC write early (degree-sorted), so
    # PE detection runs DURING the gather-bound MAC instead of after it.
    DSEQ = list(range(NDC - 1, -1, -1))
    gate_of = []
    for dc in range(NDC):
        t0, tn = _D_CHUNKS[dc]
        g = 1
        for ci, c in enumerate(calls):
            if c["cc0"] < t0 + tn and c["cc0"] + c["ncc"] > t0:
                g = ci + 1
        gate_of.append(g)
    gmax = 0
    rel_at = [0] * ncalls              # call ci -> #chunks released after it
    for k, dc in enumerate(DSEQ):
        gmax = max(gmax, gate_of[dc])
        rel_at[gmax - 1] += 1

    from contextlib import ExitStack
    es = ExitStack()
    with es:
        block = es.enter_context(nc.Block())
        sb = lambda name, shape, dt: es.enter_context(nc.sbuf_tensor(name, shape, dt))
        ps = lambda name, shape, dt: es.enter_context(nc.psum_tensor(name, shape, dt))
        sem = lambda name: es.enter_context(nc.semaphore(name))
        xTs = sb("xTs", [128, 2, KC * 128], DT16)
        Wsb = sb("Wsb", [128, KC, E], DT16)
        wdet_sb = sb("wdet_sb", [128, NT], DT16)
        acc = sb("acc", [128, NT, E], DT16)
        Cst = sb("Cst", [128, NBUF, GCC, GELEM], DT16)
        Tbuf = sb("Tbuf", [128, max(GCC * E, 4 * DCC * FC)], DT16)
        vst = sb("vst", [128, NBUF, GCC], DT16)
        vexp = sb("vexp", [128, 2, GCC, FC], DT16)
        ist = sb("ist", [128, NBUF, 256], DTI16)
        Dst = sb("Dst", [128, 2, DCC, E], DT16)
        e_sb = sb("e_sb", [128, NB * 256], DT32)
        r0_ps = ps("r0_ps", [128, 2, 512], DT32)
        e_ps = ps("e_ps", [128, NB, 256], DT32)
        s_ld = sem("s_ld"); s_xl = sem("s_xl"); s_pe_mm = sem("s_pe_mm")
        s_r0c = sem("s_r0c"); s_idx = sem("s_idx"); s_v = sem("s_v")
        s_ve_m = sem("s_ve_m"); s_accd = sem("s_accd")
        s_pe_e = sem("s_pe_e"); s_D = sem("s_D"); s_ve_dm = sem("s_ve_dm")
        s_tmpw = sem("s_tmpw"); s_efin = sem("s_efin"); s_out = sem("s_out")
        s_vx = sem("s_vx"); s_dchunk = sem("s_dchunk")
        # per-SWDGE-queue gather-completion sems (queues drain out of order)
        s_gq = [sem(f"s_g{q}") for q in range(GQ)]

        def wait_gather(eng, gc):
            """Wait for gather call `gc` (issued on queue gc%GQ) to complete."""
            eng.wait_ge(s_gq[gc % GQ], 16 * (gc // GQ + 1))

        mult = mybir.AluOpType.mult
        add = mybir.AluOpType.add
        sub = mybir.AluOpType.subtract

        TbufN = Tbuf[:, 0:GCC * E].rearrange("p (n e) -> p n e", e=E)
        Tdm = [Tbuf[:, i * (DCC * FC):(i + 1) * (DCC * FC)]
               .rearrange("p (t f) -> p t f", f=FC) for i in range(4)]

        # ----------------------------- SYNC: all HWDGE DMA -----------------
        @block.sync
        def _(sync):
            n_out = 0
            sync.dma_start(Wsb[:], W_e[:]).then_inc(s_ld, 16)
            sync.dma_start(wdet_sb[:], wdet_e[:]).then_inc(s_ld, 16)
            for g in range(NT):
                if g >= 2:
                    sync.wait_ge(s_pe_mm, g - 1)
                sync.dma_start(xTs[:, g % 2, :], xT_e[g]).then_inc(s_xl, 16)
            for b in range(max(n_tails, n_spmm + (1 if emit_e else 0)) + 1):
                if 1 <= b <= n_spmm:
                    for ci, c in enumerate(calls):
                        gc = (b - 1) * ncalls + ci
                        if do_gather and gc >= NBUF:
                            wait_gather(sync, gc - NBUF)
                        with nc.allow_non_contiguous_dma(reason="tiny idx/v slices"):
                            if do_gather:
                                sync.dma_start(ist[:, gc % NBUF, 0:c["icc"]],
                                               idx_e[:, c["ioff"]:c["ioff"] + c["icc"]]
                                               ).then_inc(s_idx, 16)
                            if do_mac:
                                if gc >= NBUF:
                                    sync.wait_ge(s_vx, gc - NBUF + 1)
                                sync.dma_start(vst[:, gc % NBUF, 0:c["ncc"]],
                                               vv_e[(b - 1) % NB][:, c["voff"]:c["voff"] + c["ncc"]]
                                               ).then_inc(s_v, 16)
                if b < n_tails:
                    # D loads + tmp writes for tail(b) (produces tmp_{b+1})
                    seq = DSEQ if (b >= 1 and do_mac) else list(range(NDC))
                    for k in range(min(2, NDC)):
                        t0, tn = _D_CHUNKS[seq[k]]
                        if b * NDC + k >= 2:
                            sync.wait_ge(s_ve_dm, b * NDC + k - 1)
                        sync.dma_start(Dst[:, k % 2, 0:tn, :],
                                       Dsp_e[:, t0:t0 + tn, :]).then_inc(s_D, 16)
                    for k in range(NDC):
                        t0, tn = _D_CHUNKS[seq[k]]
                        sync.wait_ge(s_ve_dm, b * NDC + k + 1)
                        if k + 2 < NDC:
                            t0n, tnn = _D_CHUNKS[seq[k + 2]]
                            sync.dma_start(Dst[:, (k + 2) % 2, 0:tnn, :],
                                           Dsp_e[:, t0n:t0n + tnn, :]).then_inc(s_D, 16)
                        dst = tmp[:, 0:E].rearrange("(t p) e -> p t e", p=128)
                        sync.dma_start(dst[:, t0:t0 + tn, :],
                                       acc[:, t0:t0 + tn, :]).then_inc(s_tmpw, 16)
            if emit_e:
                sync.wait_ge(s_efin, 1)
                sync.dma_start(e_out[:], e_sb[0:1, :]).then_inc(s_out, 16)
                n_out += 16
            if debug == "r0":
                sync.wait_ge(s_r0c, NT)
                sync.dma_start(dbg_e[:], acc[:]).then_inc(s_out, 16)
                n_out += 16
            elif debug == "tmp1":
                sync.wait_ge(s_tmpw, 16 * NDC)
                sync.dma_start(dbg_e[:], tmp[:]).then_inc(s_out, 16)
                n_out += 16
            elif debug in ("cur1", "cur8"):
                sync.wait_ge(s_accd, n_spmm)
                sync.dma_start(dbg_e[:], acc[:]).then_inc(s_out, 16)
                n_out += 16
            elif debug == "g8":
                for gc in range(ncalls * n_spmm - GQ, ncalls * n_spmm):
                    wait_gather(sync, gc)
                sync.dma_start(dbg_e[:], tmp[0:128, 0:16]).then_inc(s_out, 16)
                n_out += 16
            sync.wait_ge(s_out, n_out)
            if n_tails:
                sync.wait_ge(s_tmpw, 16 * NDC * n_tails)

        # ----------------------------- GPSIMD: gathers ----------------------
        @block.gpsimd
        def _(gpsimd):
            gpsimd.load_library(mlp)
            gpsimd.memset(Cst[:], 0.0)
            for b in range(1, (n_spmm if do_gather else 0) + 1):
                for ci, c in enumerate(calls):
                    gc = (b - 1) * ncalls + ci
                    gpsimd.wait_ge(s_idx, 16 * (gc + 1))
                    if do_mac and gc >= NBUF:
                        gpsimd.wait_ge(s_ve_m, gc - NBUF + 1)
                    if ci == 0:
                        gpsimd.wait_ge(s_tmpw, 16 * NDC * min(b, n_tails))
                    gpsimd.dma_gather(
                        Cst[:, gc % NBUF, 0:c["ncc"], :],
                        tmp[:, 0:GELEM],
                        ist[:, gc % NBUF, 0:c["icc"]],
                        c["nidx"], c["nidx"], GELEM,
                        elem_step=STRIDE,
                        single_packet=(c["nidx"] <= 1024),
                        queue_num=gc % GQ,
                    ).then_inc(s_gq[gc % GQ], 16)

        # ----------------------------- ACT: v-broadcast expansion -----------
        @block.scalar
        def _(scalar):
            for b in range(1, (n_spmm if do_mac else 0) + 1):
                for ci, c in enumerate(calls):
                    gc = (b - 1) * ncalls + ci
                    scalar.wait_ge(s_v, 16 * (gc + 1))
                    if gc >= 2:
                        scalar.wait_ge(s_ve_m, gc - 1)
                    ncc = c["ncc"]
                    vb = vst[:, gc % NBUF, 0:ncc].unsqueeze(-1).broadcast_to(
                        (128, ncc, FC))
                    scalar.copy(vexp[:, gc % 2, 0:ncc, :], vb).then_inc(s_vx, 1)

        # ----------------------------- VECTOR -------------------------------
        @block.vector
        def _(vector):
            for g in range(NT):
                vector.wait_ge(s_pe_mm, g + 1)
                vector.tensor_copy(acc[:, g, :], r0_ps[:, g % 2, 0:E]).then_inc(s_r0c, 1)
            for b in range(0, max(n_tails, n_spmm) + 1):
                if 1 <= b <= n_spmm and do_mac:
                    for ci, c in enumerate(calls):
                        gc = (b - 1) * ncalls + ci
                        if do_gather:
                            wait_gather(vector, gc)
                        vector.wait_ge(s_vx, gc + 1)
                        ncc, cc0 = c["ncc"], c["cc0"]
                        cst4 = Cst[:, gc % NBUF, 0:ncc, 0:E].rearrange(
                            "p c (r f) -> p c r f", f=FC)
                        vx4 = vexp[:, gc % 2, 0:ncc, :].unsqueeze(2) \
                            .broadcast_to((128, ncc, 2, FC))
                        arange = acc[:, cc0:cc0 + ncc, :]
                        if c["first"]:
                            a4 = arange.rearrange("p c (r f) -> p c r f", f=FC)
                            vector.tensor_tensor(a4, cst4, vx4, mult) \
                                .then_inc(s_ve_m, 1)
                        else:
                            t4 = TbufN[:, 0:ncc, :].rearrange(
                                "p c (r f) -> p c r f", f=FC)
                            vector.tensor_tensor(t4, cst4, vx4, mult) \
                                .then_inc(s_ve_m, 1)
                            vector.tensor_tensor(
                                arange, arange, TbufN[:, 0:ncc, :], add)
                        if rel_at[ci]:
                            # these D-chunks' rows got their last MAC write:
                            # release PE detection for them
                            vector.drain()
                            vector.sem_inc(s_dchunk, rel_at[ci])
                        if ci == ncalls - 1:
                            vector.drain()
                            vector.sem_inc(s_accd, 1)
                if b < n_tails:
                    seq = DSEQ if (b >= 1 and do_mac) else list(range(NDC))
                    for k in range(NDC):
                        t0, tn = _D_CHUNKS[seq[k]]
                        vector.wait_ge(s_D, 16 * (b * NDC + k + 1))
                        if b >= 1 and emit_e:
                            vector.wait_ge(s_pe_e, (b - 1) * NT + (NT - t0))
                        are = acc[:, t0:t0 + tn, 0:FC]
                        aim = acc[:, t0:t0 + tn, FC:E]
                        dre = Dst[:, k % 2, 0:tn, 0:FC]
                        dim = Dst[:, k % 2, 0:tn, FC:E]
                        t_ = [Tdm[i][:, 0:tn, :] for i in range(4)]
                        vector.tensor_tensor(t_[0], are, dre, mult)
                        vector.tensor_tensor(t_[1], are, dim, mult)
                        vector.tensor_tensor(t_[2], aim, dim, mult)
                        vector.tensor_tensor(t_[3], aim, dre, mult)
                        vector.tensor_tensor(are, t_[0], t_[2], sub)
                        vector.tensor_tensor(aim, t_[1], t_[3], add) \
                            .then_inc(s_ve_dm, 1)
            if emit_e:
                vector.wait_ge(s_pe_e, n_spmm * NT)
                vector.tensor_copy(
                    e_sb[0:1, :],
                    e_ps[0:1].rearrange("p b e -> p (b e)")).then_inc(s_efin, 1)

        # ----------------------------- TENSOR -------------------------------
        @block.tensor
        def _(tensor):
            tensor.wait_ge(s_ld, 32)
            for g in range(NT):
                tensor.wait_ge(s_xl, 16 * (g + 1))
                if g >= 2:
                    tensor.wait_ge(s_r0c, g - 1)
                for k in range(KC):
                    ins = tensor.matmul(
                        r0_ps[:, g % 2, 0:E],
                        xTs[:, g % 2, k * 128:(k + 1) * 128],
                        Wsb[:, k, :],
                        start=(k == 0), stop=(k == KC - 1))
                    if k == KC - 1:
                        ins.then_inc(s_pe_mm, 1)
            if emit_e:
                for b in range(1, n_spmm + 1):
                    cnt = 0
                    for k, dc in enumerate(DSEQ):
                        if do_mac:
                            tensor.wait_ge(s_dchunk, (b - 1) * NDC + k + 1)
                        else:
                            tensor.wait_ge(s_accd, b)
                        t0, tn = _D_CHUNKS[dc]
                        for t in range(t0 + tn - 1, t0 - 1, -1):
                            tensor.matmul(
                                e_ps[0:1, (b - 1) % NB, 0:E],
                                wdet_sb[:, t:t + 1],
                                acc[:, t, :],
                                start=(cnt == 0), stop=(cnt == NT - 1)
                            ).then_inc(s_pe_e, 1)
                            cnt += 1

    nc.compile()
    return nc


# ===========================================================================
# entry point
# ===========================================================================

def kernel(**inputs) -> np.ndarray:
    plan, in_maps, post = preprocess(inputs)
    nc = build_nc(plan)
    # The first execution of a freshly loaded NEFF intermittently returns
    # garbage on one core (observed under the axon/PJRT path); run up to 3
    # times until every per-core output is finite.
    out = None
    for attempt in range(3):
        res = run_bass_kernel_spmd(nc, in_maps, list(range(N_CORES)))
        if all(np.isfinite(np.asarray(r["e"])).all() for r in res.results):
            out = postprocess(res.results, post)
            break
        out = postprocess(res.results, post)
    return out


if __name__ == "__main__":
    data = dict(np.load("/root/problem/inputs_cache.npz"))
    out = kernel(**data)
    expect = np.load("/root/problem/expect_cache.npy")
    err = np.linalg.norm(out - expect) / np.linalg.norm(expect)
    print("rel err:", err)



# revision 39
# speedup vs baseline: 1.0854x; 1.0854x over previous
"""Trainium2 Bass kernel for nn_AcousticRadianceTransfer_PatchToPatch.

Strategy
--------
The 8-bounce frequency-domain loop  cur <- SpMM(vals; row,col) @ (cur * D)
is diagonal in frequency, so the 801 rfft bins are sharded over the 8 cores
(104 padded bins each) with ZERO inter-core communication.  The final output
only needs w @ total, so the big irfft collapses to a single 1600-point
irfft of the per-bounce detection spectra on the host; the R0 term reduces
to w @ initial_radiance in the time domain.

Per core:
  Phase A:  R0 = x @ W  (DFT as fp16 TensorE matmul; fsm window folded into
            W; x pre-transposed on host), written to SBUF `acc` (fp16).
  8 bounces: tmp = acc * D  ->  HBM  (fp16, rows padded to 512B pitch);
            SpMM as slot-structured dma_gather (rows degree-sorted so each
            "slot s" gather covers the prefix of rows with degree > s) +
            VectorE  acc += v * C;  per-bounce detection matvec
            e_b = w @ acc on TensorE into PSUM.
  Gathers move the 416B payload (208 fp16) from 512B-pitch rows in
  4096-index calls, staged 4-deep with one buffer per SWDGE queue
  (measured: call size and staging depth dominate SWDGE gather rate;
  82 GB/s at 5120-idx/2-deep -> 165+ GB/s at 4096-idx/4-deep).  The
  per-edge weight v is expanded along the (half-)frequency axis by the
  otherwise-idle Activation engine (v is real, so re/im halves share one
  expansion); both VectorE passes then run packed fp16 at 2x.
  Magnitudes are kept in fp16 range by per-bounce scales sigma_b folded
  into the edge weights (exact algebra, host-compensated).

Everything data-dependent (degree sort, slot plan, scales) is host-side
preprocessing; the device graph is compiled per problem instance.
"""

import math
import os
import sys

import numpy as np

if "/opt/trn_rl_repo" not in sys.path:
    sys.path.insert(0, "/opt/trn_rl_repo")

import concourse.bacc as bacc
import concourse.bass as bass
import concourse.mybir as mybir
from concourse.bass_utils import run_bass_kernel_spmd
from concourse.library_config import mlp

# ---------------- problem constants (hardcoded per contract) ----------------
SR = 16000
GAMMA = 1e-3
NB = 8                 # bounces
NRAD = 24000
L = 1600
F = L // 2 + 1         # 801
N_CORES = 8
FC = 104               # complex freqs per core (8*104 = 832 >= 801)
FPAD = N_CORES * FC    # 832
E = 2 * FC             # 208 fp16 values per radiance row slice  [re(104)|im(104)]
NROW = 24064           # 24000 padded to 188*128
NT = NROW // 128       # 188 chunk-cols
STRIDE = 256           # fp16 elems per tmp row in HBM (512B pitch)
KC = 13                # 1600 -> 13 chunks of 128 (last zero-padded)
GCC = 30               # gather staging: chunk-cols per call (<= 3840 idxs)
NBUF = 5               # gather staging depth (queues + 1: hides the MAC
                       # handoff so each SWDGE queue stays busy)
DCC = 16               # D-mult chunk-cols  (188 = 11*16 + 12)
DT16 = mybir.dt.float16
DT32 = mybir.dt.float32
DTI16 = mybir.dt.int16

_D_CHUNKS = [(i * DCC, min(DCC, NT - i * DCC)) for i in range((NT + DCC - 1) // DCC)]
NDC = len(_D_CHUNKS)   # 12


def _patch_dma_gather():
    """Relax bass's 256B elem-size assert (only the HBM row STRIDE must be a
    multiple of 256B; the payload length is arbitrary — verified on HW)."""
    import inspect
    import textwrap

    if getattr(bass.BassGpSimd.dma_gather, "_relaxed", False):
        return
    src = inspect.getsource(bass.BassGpSimd.dma_gather)
    src = src.replace(
        "elem_size_bytes > 0 and elem_size_bytes % 256 == 0",
        "elem_size_bytes > 0 and elem_size_bytes % 2 == 0",
    )
    ns = dict(bass.BassGpSimd.dma_gather.__globals__)
    exec(textwrap.dedent(src), ns)
    fn = ns["dma_gather"]
    fn._relaxed = True
    bass.BassGpSimd.dma_gather = fn


# ===========================================================================
# host preprocessing
# ===========================================================================

def _plan_calls(ndeg):
    """Static gather-call plan from the (descending) degree array."""
    maxdeg = int(ndeg.max())
    counts = np.bincount(ndeg, minlength=maxdeg + 2)
    # Ns[s] = #rows with degree > s  (rows sorted desc -> prefix property)
    tail = np.cumsum(counts[::-1])[::-1]
    calls = []
    for s in range(maxdeg):
        Ns = int(tail[s + 1])
        if Ns <= 0:
            break
        ncc_s = (Ns + 127) // 128
        for cc0 in range(0, ncc_s, GCC):
            ncc = min(GCC, ncc_s - cc0)
            base = cc0 * 128
            nidx = min(Ns - base, ncc * 128)
            calls.append(dict(slot=s, cc0=cc0, ncc=ncc, nidx=nidx,
                              first=(s == 0)))
    return calls


def _pack_idx(idx_vals, icc):
    """int16 [128, icc]: wrapped in 16 partitions, replicated across 8 Q7 cores."""
    n = len(idx_vals)
    arr = np.full((icc, 16), -1, np.int16)
    arr.reshape(-1)[:n] = idx_vals             # position i -> [i//16, i%16]
    return np.tile(arr.T, (8, 1))              # -> [p=i%16, col=i//16]


def preprocess(inputs):
    """All host-side data preparation.  Returns (plan, in_maps, post)."""
    a = np.asarray(inputs["absorption_coefficient"], np.float64)
    s = np.asarray(inputs["scattering_coefficient"], np.float64)
    x = np.asarray(inputs["initial_radiance"], np.float32)
    wdet = np.asarray(inputs["detection_weights"], np.float64)
    env = np.asarray(inputs["envelope"], np.float64)
    basis = np.asarray(inputs["sparse_kernel_basis"], np.float64)
    row = np.asarray(inputs["sparse_kernel_row"]).astype(np.int64)
    col = np.asarray(inputs["sparse_kernel_col"]).astype(np.int64)
    refl = np.asarray(inputs["sparse_kernel_reflector_id"]).astype(np.int64)
    dly = np.asarray(inputs["delay_samples"], np.float64)

    log_gamma = math.log(GAMMA)
    t = np.arange(L) / SR
    fsm = np.exp(log_gamma * t)

    vals = (s[:, refl] * basis).sum(0) * a[refl]          # [nnz]

    # --- degree sort / relabel (dummy edge for degree-0 rows) ---
    deg = np.bincount(row, minlength=NRAD)
    zrows = np.nonzero(deg == 0)[0]
    if len(zrows):
        row = np.concatenate([row, zrows])
        col = np.concatenate([col, np.zeros(len(zrows), np.int64)])
        vals = np.concatenate([vals, np.zeros(len(zrows))])
        deg = np.bincount(row, minlength=NRAD)
    order = np.argsort(-deg, kind="stable")               # old rows, deg desc
    new_of_old = np.empty(NRAD, np.int64)
    new_of_old[order] = np.arange(NRAD)
    nrow = new_of_old[row]
    ncol = new_of_old[col]
    ndeg = deg[order]                                     # degree by new label

    # --- per-bounce scales from the exact DC trajectory (upper bound) ---
    D0 = np.exp(log_gamma * dly / SR)
    cur_dc = (x.astype(np.float64) * fsm).sum(1)
    sig = []
    for b in range(NB):
        nxt = np.zeros(NRAD)
        np.add.at(nxt, row, vals * (D0 * cur_dc)[col])
        g = max(nxt.max(), 1e-30) / 256.0
        sig.append(g)
        cur_dc = nxt / g
    sig = np.array(sig)
    alpha = np.cumprod(sig)

    # --- slot structure: edges sorted by (new row, position) ---
    nnz = len(vals)
    eorder = np.lexsort((np.arange(nnz), nrow))
    erow = nrow[eorder]
    ecol = ncol[eorder]
    ev = vals[eorder]
    starts = np.zeros(NRAD + 1, np.int64)
    np.cumsum(np.bincount(erow, minlength=NRAD), out=starts[1:])

    calls = _plan_calls(ndeg)

    idx_blocks = []
    v_blocks = [[] for _ in range(NB)]
    for c in calls:
        sslot, base, nidx, ncc = c["slot"], c["cc0"] * 128, c["nidx"], c["ncc"]
        eidx = starts[base:base + nidx] + sslot
        icc = (nidx + 15) // 16
        c["icc"] = icc
        idx_blocks.append(_pack_idx(ecol[eidx].astype(np.int16), icc))
        vraw = np.zeros(ncc * 128, np.float64)
        vraw[:nidx] = ev[eidx]
        vgrid = vraw.reshape(ncc, 128).T                  # [128, ncc]
        for b in range(NB):
            v_blocks[b].append((vgrid / sig[b]).astype(np.float16))
    idx_all = np.concatenate(idx_blocks, axis=1)          # [128, IT]
    vv_all = np.stack([np.concatenate(vb, axis=1) for vb in v_blocks])  # [NB,128,VT]

    # --- DFT operands ---
    xp = np.zeros((NROW, KC * 128), np.float16)
    xp[new_of_old, :L] = x.astype(np.float16)
    xT = np.ascontiguousarray(
        xp.reshape(NT, 128, KC, 128).transpose(0, 3, 2, 1)
    ).reshape(NT, 128, KC * 128)                          # [g, time, k*128+r]

    k_all = np.arange(FPAD)
    # W[t, k] = fsm[t] * exp(-2i pi t k / L); zero for k >= F and t >= L
    Wfull = np.zeros((KC * 128, FPAD), np.complex128)
    Wfull[:L, :F] = np.exp(-2j * np.pi * np.outer(np.arange(L), k_all[:F]) / L) \
        * fsm[:, None]

    Dc = np.exp(log_gamma * dly / SR)[:, None] * \
        np.exp(-2j * np.pi * np.outer(dly, k_all) / L)    # [NRAD, FPAD]
    Dc[:, F:] = 0.0

    wperm = np.zeros(NROW)
    wperm[new_of_old] = wdet
    wdet_sb = wperm.reshape(NT, 128).T.astype(np.float16)  # [128, NT]

    in_maps = []
    for cidx in range(N_CORES):
        ks = slice(cidx * FC, (cidx + 1) * FC)
        Wc = np.empty((KC, 128, E), np.float16)
        Wslice = Wfull[:, ks]
        Wc[:, :, :FC] = Wslice.real.reshape(KC, 128, FC)
        Wc[:, :, FC:] = Wslice.imag.reshape(KC, 128, FC)
        Wc = np.ascontiguousarray(Wc.transpose(1, 0, 2))  # [128, KC, E]
        Dslice = Dc[:, ks]
        Dacc = np.zeros((128, NT, E), np.float16)
        Dre = np.zeros((NROW, FC)); Dim = np.zeros((NROW, FC))
        Dre[new_of_old] = Dslice.real
        Dim[new_of_old] = Dslice.imag
        Dacc[:, :, :FC] = Dre.reshape(NT, 128, FC).transpose(1, 0, 2)
        Dacc[:, :, FC:] = Dim.reshape(NT, 128, FC).transpose(1, 0, 2)
        in_maps.append({
            "xT": xT,
            "W": Wc,
            "wdet": wdet_sb,
            "Dsp": Dacc.astype(np.float16),
            "vv": vv_all,
            "idx": idx_all,
        })

    post = dict(alpha=alpha, fsm=fsm, env=env,
                e0=float(0.0) * np.zeros(1),  # placeholder
                )
    post["e0"] = wdet @ x.astype(np.float64)              # [L]
    plan = dict(calls=calls, IT=idx_all.shape[1], VT=vv_all.shape[2])
    return plan, in_maps, post


def postprocess(results, post):
    """Assemble echogram from per-core per-bounce detection spectra."""
    alpha, fsm, env, e0 = post["alpha"], post["fsm"], post["env"], post["e0"]
    e_spec = np.zeros(FPAD, np.complex128)
    for cidx in range(N_CORES):
        eb = np.asarray(results[cidx]["e"], np.float64).reshape(NB, 256)
        ks = slice(cidx * FC, (cidx + 1) * FC)
        acc = np.zeros(FC, np.complex128)
        for b in range(NB):
            acc += alpha[b] * (eb[b, :FC] + 1j * eb[b, FC:E])
        e_spec[ks] = acc
    echo_b = np.fft.irfft(e_spec[:F], n=L) / fsm
    echo = (e0 + echo_b) * np.exp(env)
    return echo.astype(np.float32)


# ===========================================================================
# device graph
# ===========================================================================

def build_nc(plan, debug=None):
    """debug: None (full), "r0" (phase A only), "tmp1" (+tail0), "cur1"
    (+bounce1 SpMM), "cur8" (all SpMM, dump acc after bounce 8)."""
    calls = plan["calls"]
    ncalls = len(calls)
    IT, VT = plan["IT"], plan["VT"]

    # how much of the pipeline to emit
    if debug == "r0":
        n_tails, n_spmm, emit_e = 0, 0, False
    elif debug == "tmp1":
        n_tails, n_spmm, emit_e = 1, 0, False
    elif debug == "cur1":
        n_tails, n_spmm, emit_e = 1, 1, False
    elif debug == "cur8":
        n_tails, n_spmm, emit_e = NB, NB, False
    elif debug == "b4":
        n_tails, n_spmm, emit_e = 4, 4, True
    elif debug == "b1":
        n_tails, n_spmm, emit_e = 1, 1, True
    elif debug == "b0":
        n_tails, n_spmm, emit_e = 1, 0, True
    elif debug == "b16":
        n_tails, n_spmm, emit_e = 16, 16, True
    else:
        n_tails, n_spmm, emit_e = NB, NB, True
    # timing-isolation variants: g8 = gathers only; m8 = all but gathers
    do_gather = debug != "m8"
    do_mac = debug != "g8"
    if debug == "g8":
        n_tails, n_spmm, emit_e = 1, NB, False
    elif debug == "m8":
        n_tails, n_spmm, emit_e = NB, NB, True

    # gather experiment knobs (env): payload elems per row, #SWDGE queues
    GELEM = int(os.environ.get("GELEM", "208"))
    GQ = int(os.environ.get("GQ", "4"))
    if (GELEM * 2) % 256:
        _patch_dma_gather()

    nc = bacc.Bacc("TRN2", num_swdge_queues=GQ,
                   dynamic_dma_scratch_size=int(os.environ.get("DSCR", "24576")))

    xT_e = nc.declare_dram_parameter("xT", [NT, 128, KC * 128], DT16, isOutput=False)
    W_e = nc.declare_dram_parameter("W", [128, KC, E], DT16, isOutput=False)
    wdet_e = nc.declare_dram_parameter("wdet", [128, NT], DT16, isOutput=False)
    Dsp_e = nc.declare_dram_parameter("Dsp", [128, NT, E], DT16, isOutput=False)
    vv_e = nc.declare_dram_parameter("vv", [NB, 128, VT], DT16, isOutput=False)
    idx_e = nc.declare_dram_parameter("idx", [128, IT], DTI16, isOutput=False)
    e_out = None
    if emit_e:
        e_out = nc.declare_dram_parameter("e", [1, NB * 256], DT32, isOutput=True)
    dbg_e = None
    if debug in ("r0", "cur1", "cur8"):
        dbg_e = nc.declare_dram_parameter("dbg", [128, NT, E], DT16, isOutput=True)
    elif debug == "tmp1":
        dbg_e = nc.declare_dram_parameter("dbg", [NROW, STRIDE], DT16, isOutput=True)
    elif debug == "g8":
        dbg_e = nc.declare_dram_parameter("dbg", [128, 16], DT16, isOutput=True)

    tmp = nc.dram_tensor("tmp", [NROW, STRIDE], DT16)

    ioff = 0
    voff = 0
    for c in calls:
        c["ioff"] = ioff
        c["voff"] = voff
        ioff += c["icc"]
        voff += c["ncc"]
    assert ioff == IT and voff == VT

    # ---- per-chunk detection release: process D-chunks in descending-t ----
    # order; chunk rows get their last MAC write early (degree-sorted), so
    # PE detection runs DURING the gather-bound MAC instead of after it.
    DSEQ = list(range(NDC - 1, -1, -1))
    gate_of = []
    for dc in range(NDC):
        t0, tn = _D_CHUNKS[dc]
        g = 1
        for ci, c in enumerate(calls):
            if c["cc0"] < t0 + tn and c["cc0"] + c["ncc"] > t0:
                g = ci + 1
        gate_of.append(g)
    gmax = 0
    rel_at = [0] * ncalls              # call ci -> #chunks released after it
    for k, dc in enumerate(DSEQ):
        gmax = max(gmax, gate_of[dc])
        rel_at[gmax - 1] += 1

    from contextlib import ExitStack
    es = ExitStack()
    with es:
        block = es.enter_context(nc.Block())
        sb = lambda name, shape, dt: es.enter_context(nc.sbuf_tensor(name, shape, dt))
        ps = lambda name, shape, dt: es.enter_context(nc.psum_tensor(name, shape, dt))
        sem = lambda name: es.enter_context(nc.semaphore(name))
        xTs = sb("xTs", [128, 2, KC * 128], DT16)
        Wsb = sb("Wsb", [128, KC, E], DT16)
        wdet_sb = sb("wdet_sb", [128, NT], DT16)
        acc = sb("acc", [128, NT, E], DT16)
        Cst = sb("Cst", [128, NBUF, GCC, GELEM], DT16)
        Tbuf = sb("Tbuf", [128, max(GCC * E, 4 * DCC * FC)], DT16)
        vst = sb("vst", [128, NBUF, GCC], DT16)
        vexp = sb("vexp", [128, 2, GCC, FC], DT16)
        ist = sb("ist", [128, NBUF, 256], DTI16)
        Dst = sb("Dst", [128, 2, DCC, E], DT16)
        e_sb = sb("e_sb", [128, NB * 256], DT32)
        r0_ps = ps("r0_ps", [128, 2, 512], DT32)
        e_ps = ps("e_ps", [128, NB, 256], DT32)
        s_ld = sem("s_ld"); s_xl = sem("s_xl"); s_pe_mm = sem("s_pe_mm")
        s_r0c = sem("s_r0c"); s_idx = sem("s_idx"); s_v = sem("s_v")
        s_ve_m = sem("s_ve_m"); s_accd = sem("s_accd")
        s_pe_e = sem("s_pe_e"); s_D = sem("s_D"); s_ve_dm = sem("s_ve_dm")
        s_tmpw = sem("s_tmpw"); s_efin = sem("s_efin"); s_out = sem("s_out")
        s_vx = sem("s_vx"); s_dchunk = sem("s_dchunk")
        # per-SWDGE-queue gather-completion sems (queues drain out of order)
        s_gq = [sem(f"s_g{q}") for q in range(GQ)]

        def wait_gather(eng, gc):
            """Wait for gather call `gc` (issued on queue gc%GQ) to complete."""
            eng.wait_ge(s_gq[gc % GQ], 16 * (gc // GQ + 1))

        mult = mybir.AluOpType.mult
        add = mybir.AluOpType.add
        sub = mybir.AluOpType.subtract

        TbufN = Tbuf[:, 0:GCC * E].rearrange("p (n e) -> p n e", e=E)
        Tdm = [Tbuf[:, i * (DCC * FC):(i + 1) * (DCC * FC)]
               .rearrange("p (t f) -> p t f", f=FC) for i in range(4)]

        # ----------------------------- SYNC: all HWDGE DMA -----------------
        @block.sync
        def _(sync):
            n_out = 0
            sync.dma_start(Wsb[:], W_e[:]).then_inc(s_ld, 16)
            sync.dma_start(wdet_sb[:], wdet_e[:]).then_inc(s_ld, 16)
            for g in range(NT):
                if g >= 2:
                    sync.wait_ge(s_pe_mm, g - 1)
                sync.dma_start(xTs[:, g % 2, :], xT_e[g]).then_inc(s_xl, 16)
            for b in range(max(n_tails, n_spmm + (1 if emit_e else 0)) + 1):
                if 1 <= b <= n_spmm:
                    for ci, c in enumerate(calls):
                        gc = (b - 1) * ncalls + ci
                        if do_gather and gc >= NBUF:
                            wait_gather(sync, gc - NBUF)
                        with nc.allow_non_contiguous_dma(reason="tiny idx/v slices"):
                            if do_gather:
                                sync.dma_start(ist[:, gc % NBUF, 0:c["icc"]],
                                               idx_e[:, c["ioff"]:c["ioff"] + c["icc"]]
                                               ).then_inc(s_idx, 16)
                            if do_mac:
                                if gc >= NBUF:
                                    sync.wait_ge(s_vx, gc - NBUF + 1)
                                sync.dma_start(vst[:, gc % NBUF, 0:c["ncc"]],
                                               vv_e[(b - 1) % NB][:, c["voff"]:c["voff"] + c["ncc"]]
                                               ).then_inc(s_v, 16)
                if b < n_tails:
                    # D loads + tmp writes for tail(b) (produces tmp_{b+1})
                    seq = DSEQ if (b >= 1 and do_mac) else list(range(NDC))
                    for k in range(min(2, NDC)):
                        t0, tn = _D_CHUNKS[seq[k]]
                        if b * NDC + k >= 2:
                            sync.wait_ge(s_ve_dm, b * NDC + k - 1)
                        sync.dma_start(Dst[:, k % 2, 0:tn, :],
                                       Dsp_e[:, t0:t0 + tn, :]).then_inc(s_D, 16)
                    for k in range(NDC):
                        t0, tn = _D_CHUNKS[seq[k]]
                        sync.wait_ge(s_ve_dm, b * NDC + k + 1)
                        if k + 2 < NDC:
                            t0n, tnn = _D_CHUNKS[seq[k + 2]]
                            sync.dma_start(Dst[:, (k + 2) % 2, 0:tnn, :],
                                           Dsp_e[:, t0n:t0n + tnn, :]).then_inc(s_D, 16)
                        dst = tmp[:, 0:E].rearrange("(t p) e -> p t e", p=128)
                        sync.dma_start(dst[:, t0:t0 + tn, :],
                                       acc[:, t0:t0 + tn, :]).then_inc(s_tmpw, 16)
            if emit_e:
                sync.wait_ge(s_efin, 1)
                sync.dma_start(e_out[:], e_sb[0:1, :]).then_inc(s_out, 16)
                n_out += 16
            if debug == "r0":
                sync.wait_ge(s_r0c, NT)
                sync.dma_start(dbg_e[:], acc[:]).then_inc(s_out, 16)
                n_out += 16
            elif debug == "tmp1":
                sync.wait_ge(s_tmpw, 16 * NDC)
                sync.dma_start(dbg_e[:], tmp[:]).then_inc(s_out, 16)
                n_out += 16
            elif debug in ("cur1", "cur8"):
                sync.wait_ge(s_accd, n_spmm)
                sync.dma_start(dbg_e[:], acc[:]).then_inc(s_out, 16)
                n_out += 16
            elif debug == "g8":
                for gc in range(ncalls * n_spmm - GQ, ncalls * n_spmm):
                    wait_gather(sync, gc)
                sync.dma_start(dbg_e[:], tmp[0:128, 0:16]).then_inc(s_out, 16)
                n_out += 16
            sync.wait_ge(s_out, n_out)
            if n_tails:
                sync.wait_ge(s_tmpw, 16 * NDC * n_tails)

        # ----------------------------- GPSIMD: gathers ----------------------
        @block.gpsimd
        def _(gpsimd):
            gpsimd.load_library(mlp)
            gpsimd.memset(Cst[:], 0.0)
            for b in range(1, (n_spmm if do_gather else 0) + 1):
                for ci, c in enumerate(calls):
                    gc = (b - 1) * ncalls + ci
                    gpsimd.wait_ge(s_idx, 16 * (gc + 1))
                    if do_mac and gc >= NBUF:
                        gpsimd.wait_ge(s_ve_m, gc - NBUF + 1)
                    if ci == 0:
                        gpsimd.wait_ge(s_tmpw, 16 * NDC * min(b, n_tails))
                    gpsimd.dma_gather(
                        Cst[:, gc % NBUF, 0:c["ncc"], :],
                        tmp[:, 0:GELEM],
                        ist[:, gc % NBUF, 0:c["icc"]],
                        c["nidx"], c["nidx"], GELEM,
                        elem_step=STRIDE,
                        single_packet=(c["nidx"] <= 1024),
                        queue_num=gc % GQ,
                    ).then_inc(s_gq[gc % GQ], 16)

        # ----------------------------- ACT: v-broadcast expansion -----------
        @block.scalar
        def _(scalar):
            for b in range(1, (n_spmm if do_mac else 0) + 1):
                for ci, c in enumerate(calls):
                    gc = (b - 1) * ncalls + ci
                    scalar.wait_ge(s_v, 16 * (gc + 1))
                    if gc >= 2:
                        scalar.wait_ge(s_ve_m, gc - 1)
                    ncc = c["ncc"]
                    vb = vst[:, gc % NBUF, 0:ncc].unsqueeze(-1).broadcast_to(
                        (128, ncc, FC))
                    scalar.copy(vexp[:, gc % 2, 0:ncc, :], vb).then_inc(s_vx, 1)

        # ----------------------------- VECTOR -------------------------------
        @block.vector
        def _(vector):
            for g in range(NT):
                vector.wait_ge(s_pe_mm, g + 1)
                vector.tensor_copy(acc[:, g, :], r0_ps[:, g % 2, 0:E]).then_inc(s_r0c, 1)
            for b in range(0, max(n_tails, n_spmm) + 1):
                if 1 <= b <= n_spmm and do_mac:
                    for ci, c in enumerate(calls):
                        gc = (b - 1) * ncalls + ci
                        if do_gather:
                            wait_gather(vector, gc)
                        vector.wait_ge(s_vx, gc + 1)
                        ncc, cc0 = c["ncc"], c["cc0"]
                        cst4 = Cst[:, gc % NBUF, 0:ncc, 0:E].rearrange(
                            "p c (r f) -> p c r f", f=FC)
                        vx4 = vexp[:, gc % 2, 0:ncc, :].unsqueeze(2) \
                            .broadcast_to((128, ncc, 2, FC))
                        arange = acc[:, cc0:cc0 + ncc, :]
                        if c["first"]:
                            a4 = arange.rearrange("p c (r f) -> p c r f", f=FC)
                            vector.tensor_tensor(a4, cst4, vx4, mult) \
                                .then_inc(s_ve_m, 1)
                        else:
                            t4 = TbufN[:, 0:ncc, :].rearrange(
                                "p c (r f) -> p c r f", f=FC)
                            vector.tensor_tensor(t4, cst4, vx4, mult) \
                                .then_inc(s_ve_m, 1)
                            vector.tensor_tensor(
                                arange, arange, TbufN[:, 0:ncc, :], add)
                        if rel_at[ci]:
                            # these D-chunks' rows got their last MAC write:
                            # release PE detection for them
                            vector.drain()
                            vector.sem_inc(s_dchunk, rel_at[ci])
                        if ci == ncalls - 1:
                            vector.drain()
                            vector.sem_inc(s_accd, 1)
                if b < n_tails:
                    seq = DSEQ if (b >= 1 and do_mac) else list(range(NDC))
                    for k in range(NDC):
                        t0, tn = _D_CHUNKS[seq[k]]
                        vector.wait_ge(s_D, 16 * (b * NDC + k + 1))
                        if b >= 1 and emit_e:
                            vector.wait_ge(s_pe_e, (b - 1) * NT + (NT - t0))
                        are = acc[:, t0:t0 + tn, 0:FC]
                        aim = acc[:, t0:t0 + tn, FC:E]
                        dre = Dst[:, k % 2, 0:tn, 0:FC]
                        dim = Dst[:, k % 2, 0:tn, FC:E]
                        t_ = [Tdm[i][:, 0:tn, :] for i in range(4)]
                        vector.tensor_tensor(t_[0], are, dre, mult)
                        vector.tensor_tensor(t_[1], are, dim, mult)
                        vector.tensor_tensor(t_[2], aim, dim, mult)
                        vector.tensor_tensor(t_[3], aim, dre, mult)
                        vector.tensor_tensor(are, t_[0], t_[2], sub)
                        vector.tensor_tensor(aim, t_[1], t_[3], add) \
                            .then_inc(s_ve_dm, 1)
            if emit_e:
                vector.wait_ge(s_pe_e, n_spmm * NT)
                vector.tensor_copy(
                    e_sb[0:1, :],
                    e_ps[0:1].rearrange("p b e -> p (b e)")).then_inc(s_efin, 1)

        # ----------------------------- TENSOR -------------------------------
        @block.tensor
        def _(tensor):
            tensor.wait_ge(s_ld, 32)
            for g in range(NT):
                tensor.wait_ge(s_xl, 16 * (g + 1))
                if g >= 2:
                    tensor.wait_ge(s_r0c, g - 1)
                for k in range(KC):
                    ins = tensor.matmul(
                        r0_ps[:, g % 2, 0:E],
                        xTs[:, g % 2, k * 128:(k + 1) * 128],
                        Wsb[:, k, :],
                        start=(k == 0), stop=(k == KC - 1))
                    if k == KC - 1:
                        ins.then_inc(s_pe_mm, 1)
            if emit_e:
                for b in range(1, n_spmm + 1):
                    cnt = 0
                    for k, dc in enumerate(DSEQ):
                        if do_mac:
                            tensor.wait_ge(s_dchunk, (b - 1) * NDC + k + 1)
                        else:
                            tensor.wait_ge(s_accd, b)
                        t0, tn = _D_CHUNKS[dc]
                        for t in range(t0 + tn - 1, t0 - 1, -1):
                            tensor.matmul(
                                e_ps[0:1, (b - 1) % NB, 0:E],
                                wdet_sb[:, t:t + 1],
                                acc[:, t, :],
                                start=(cnt == 0), stop=(cnt == NT - 1)
                            ).then_inc(s_pe_e, 1)
                            cnt += 1

    nc.compile()
    return nc


# ===========================================================================
# entry point
# ===========================================================================

def kernel(**inputs) -> np.ndarray:
    plan, in_maps, post = preprocess(inputs)
    nc = build_nc(plan)
    # The first execution of a freshly loaded NEFF intermittently returns
    # garbage on one core (observed under the axon/PJRT path); run up to 3
    # times until every per-core output is finite.
    out = None
    for attempt in range(3):
        res = run_bass_kernel_spmd(nc, in_maps, list(range(N_CORES)))
        if all(np.isfinite(np.asarray(r["e"])).all() for r in res.results):
            out = postprocess(res.results, post)
            break
        out = postprocess(res.results, post)
    return out


if __name__ == "__main__":
    data = dict(np.load("/root/problem/inputs_cache.npz"))
    out = kernel(**data)
    expect = np.load("/root/problem/expect_cache.npy")
    err = np.linalg.norm(out - expect) / np.linalg.norm(expect)
    print("rel err:", err)

